# revision 2
# baseline (speedup 1.0000x reference)
"""GNN message passing (GraphConv_CA) kernel for Trainium2 (8 NeuronCores).

Problem: embed [50000, 64] f32; edge_index [2, 800000] i64; trend [800000] f32.
Per hop (x3): msg = agg[row] * trend; agg = segment_sum(msg, col, N).
Output: [50000, 4, 64] = concat(embed, hop1, hop2, hop3) along axis 1.

Strategy (scatter-free dual-grid padded gather):
  SWDGE descriptor generation for dma_gather/dma_scatter_add runs on just
  2 of 8 Q7 cores (~7-8 ns/index, serialized on the Pool engine) and is
  the hard bottleneck.  The baseline paid it twice per edge (gather +
  scatter, ~209k idx/hop/core).  Here every edge is gathered DIRECTLY
  into its combine slot (~110k idx/hop/core) and the per-node sum is a
  strided DVE tensor_reduce; there are no scatters.

  - Col-sharding: core c owns dst nodes [c*6250, (c+1)*6250), padded to
    6272 = 49 groups x 128 lanes.
  - int16 gather indices address <=32768 rows, so sources are read via a
    lo view (rows [0, 32768)) or hi view (rows [17408, 50176)).  A single
    grid padded to max(#lo)+max(#hi) per group wastes ~60%, so two
    INDEPENDENT grids are used, each tight under its own sort:
      lo grid: nodes sorted by #lo-sources;  defines the P layout.
      hi grid: nodes sorted by #hi-sources (internal order).
    Each node's messages land at (partition = lane, channels of its
    group); pad slots gather row 0 with trend 0.  Per-group-run
    tensor_reduce over the W channels yields comb_lo (P order) and
    comb_hi (hi order).  comb_hi is dumped to DRAM and re-gathered in P
    order (6272 extra indices) and added to comb_lo.
  - P layout: agg row o*6272 + j*49 + g holds core o's node at lo-sorted
    position g*128+j, so the [128, 49, 64] combine tile dumps contiguously
    to DRAM, the AllGather concatenates core blocks, and the host undoes
    the permutation once at the end.
  - Per hop: hi chunks gather/reduce first, then lo chunks, then the perm
    gather (all Pool work back to back); DVE multiply/reduce overlaps the
    next chunk's gather.  Hops 1,2 AllGather the [6272, 64] partials.
"""

import sys

sys.path.insert(0, "/opt/trn_rl_repo")

import numpy as np

import concourse.bacc as bacc
import concourse.mybir as mybir
import concourse.tile as tile
from concourse.bass_utils import run_bass_kernel_spmd

F32 = mybir.dt.float32
I16 = mybir.dt.int16

N = 50000
E = 800000
D = 64
HOPS = 3
NCORES = 8
NLOC = N // NCORES          # 6250
NGRP = -(-NLOC // 128)      # 49
NLOCP = NGRP * 128          # 6272 padded nodes per core
NP = NCORES * NLOCP         # 50176 padded rows
RS = 5 * NLOCP              # 31360 rows in lo view (cores 0-4) <= 32768
HI_OFF = NP - 32768         # 17408; hi view = rows [17408, 50176)
CH_CAP = 64                 # channels per dma_gather (<=15360 idxs)
LAST_CAP = 28               # small final lo chunk to shrink the hop tail
SINGLE_PACKET = False
N_PREP = 0                  # hi chunks prepped ahead across hop boundaries


class Plan:
    pass


def _wrap16(arr):
    """[L] int16 -> [128, L//16]: position i at (i%16, i//16), replicated
    to 8 groups of 16 partitions (SWDGE index-ring layout)."""
    w = arr.reshape(-1, 16).T
    return np.tile(w, (8, 1)).copy()


def _chunk_plan(W):
    """Greedy-pack groups into chunks of <= CH_CAP channels.
    Returns list of (g0, g1, ch0, nch) with ch0 = cumulative channel base,
    plus per-chunk runs [(goff, k, Wval, ch_off_local)]."""
    chunks = []
    g0, ch0 = 0, 0
    while g0 < len(W):
        g1, s = g0, 0
        while g1 < len(W) and s + W[g1] <= CH_CAP:
            s += int(W[g1])
            g1 += 1
        assert g1 > g0, (g0, W[g0])
        chunks.append((g0, g1, ch0, int(s)))
        ch0 += int(s)
        g0 = g1
    runs = []
    for (g0, g1, _, _) in chunks:
        rr, i, off = [], g0, 0
        while i < g1:
            k = i
            while k < g1 and W[k] == W[i]:
                k += 1
            if W[i] > 0:
                rr.append((i - g0, k - i, int(W[i]), off))
            off += int(W[i]) * (k - i)
            i = k
        runs.append(rr)
    return chunks, runs


def _chunk_plan_runs(W, chunk):
    g0, g1, _, _ = chunk
    rr, i, off = [], g0, 0
    while i < g1:
        k = i
        while k < g1 and W[k] == W[i]:
            k += 1
        if W[i] > 0:
            rr.append((i - g0, k - i, int(W[i]), off))
        off += int(W[i]) * (k - i)
        i = k
    return rr


def preprocess(embed, edge_index, trend):
    row = np.asarray(edge_index[0], dtype=np.int64)
    col = np.asarray(edge_index[1], dtype=np.int64)
    trend = np.asarray(trend, dtype=np.float32)
    plan = Plan()
    pos2row = (np.arange(NLOCP) % 128) * NGRP + (np.arange(NLOCP) // 128)

    cores = []
    for c in range(NCORES):
        m = (col // NLOC) == c
        cores.append((row[m], col[m] - c * NLOC, trend[m]))

    def make_P(orders):
        P = np.zeros(N, np.int64)
        for c in range(NCORES):
            inv = np.empty(NLOCP, np.int64)
            inv[orders[c]] = np.arange(NLOCP)
            P[c * NLOC:(c + 1) * NLOC] = c * NLOCP + pos2row[inv[:NLOC]]
        return P

    # lo <=> source owned by cores 0-4 (P rows [0, 31360)); depends only on
    # the raw source id, so one sorting pass suffices.
    nlo = np.zeros((NCORES, NLOCP), np.int64)
    nhi = np.zeros((NCORES, NLOCP), np.int64)
    lo_order, hi_order = [], []
    for c in range(NCORES):
        r, cl, t = cores[c]
        lo = (r // NLOC) < 5
        nlo[c] = np.bincount(cl[lo], minlength=NLOCP)
        nhi[c] = np.bincount(cl[~lo], minlength=NLOCP)
        lo_order.append(np.lexsort((np.arange(NLOCP), -nlo[c])))
        hi_order.append(np.lexsort((np.arange(NLOCP), -nhi[c])))
    P = make_P(lo_order)
    WloG = np.zeros(NGRP, np.int64)
    WhiG = np.zeros(NGRP, np.int64)
    for c in range(NCORES):
        WloG = np.maximum(WloG, nlo[c][lo_order[c]].reshape(NGRP, 128).max(1))
        WhiG = np.maximum(WhiG, nhi[c][hi_order[c]].reshape(NGRP, 128).max(1))
    WloG = np.maximum(WloG, 1)
    WhiG = np.maximum(WhiG, 1)

    plan.WloG, plan.WhiG = WloG, WhiG
    plan.lo_chunks, plan.lo_runs = _chunk_plan(WloG)
    plan.hi_chunks, plan.hi_runs = _chunk_plan(WhiG)
    # re-chunk the lo tail so the final chunk is small (shorter hop tail)
    g0l, g1l, ch0l, nchl = plan.lo_chunks[-1]
    if nchl > LAST_CAP and g1l - g0l >= 2:
        gs, s = g1l, 0
        while gs > g0l and s + WloG[gs - 1] <= LAST_CAP:
            gs -= 1
            s += int(WloG[gs])
        if g0l < gs < g1l:
            head = (g0l, gs, ch0l, int(nchl - s))
            tail = (gs, g1l, int(ch0l + nchl - s), int(s))
            plan.lo_chunks = plan.lo_chunks[:-1] + [head, tail]
            plan.lo_runs = plan.lo_runs[:-1] + [
                _chunk_plan_runs(WloG, head), _chunk_plan_runs(WloG, tail)]
    plan.NLO_CH = int(WloG.sum())
    plan.NHI_CH = int(WhiG.sum())
    plan.CH_TOT = plan.NLO_CH + plan.NHI_CH
    plan.CHMAX = max(nch for _, _, _, nch in plan.lo_chunks + plan.hi_chunks)
    plan.order = lo_order
    plan.P = P

    # per-core input arrays
    in_maps = []
    embed_perm = np.zeros((NP, D), np.float32)
    emb = np.asarray(embed, dtype=np.float32)
    for c in range(NCORES):
        valid = np.where(lo_order[c] < NLOC)[0]
        embed_perm[c * NLOCP + pos2row[valid]] = emb[c * NLOC +
                                                     lo_order[c][valid]]

    # channel bases per group within each stream
    lo_base = np.zeros(NGRP, np.int64)
    hi_base = np.zeros(NGRP, np.int64)
    lo_base[1:] = np.cumsum(WloG)[:-1]
    hi_base[1:] = np.cumsum(WhiG)[:-1]

    for c in range(NCORES):
        r, cl, t, = cores[c]
        inv_lo = np.empty(NLOCP, np.int64)
        inv_lo[lo_order[c]] = np.arange(NLOCP)
        inv_hi = np.empty(NLOCP, np.int64)
        inv_hi[hi_order[c]] = np.arange(NLOCP)
        srow = P[r]
        lo = srow < RS

        idx_lo = np.zeros(128 * plan.NLO_CH, np.int16)
        idx_hi = np.zeros(128 * plan.NHI_CH, np.int16)
        trd = np.zeros(128 * plan.CH_TOT, np.float32)

        for kind in (0, 1):
            sel = np.where(lo if kind == 0 else ~lo)[0]
            pn = (inv_lo if kind == 0 else inv_hi)[cl[sel]]
            o = np.lexsort((sel, pn))
            sel, pn = sel[o], pn[o]
            # rank within node
            uniq, starts, cnts = np.unique(pn, return_index=True,
                                           return_counts=True)
            w = np.arange(len(pn)) - np.repeat(starts, cnts)
            g, j = pn // 128, pn % 128
            if kind == 0:
                ch = lo_base[g] + w
                idx_lo[ch * 128 + j] = srow[sel].astype(np.int16)
                trd[ch * 128 + j] = t[sel]
            else:
                ch = hi_base[g] + w
                idx_hi[ch * 128 + j] = (srow[sel] - HI_OFF).astype(np.int16)
                trd[(plan.NLO_CH + ch) * 128 + j] = t[sel]

        # perm: lo-pos p -> hi row of the same node
        nid = lo_order[c]                       # node at lo-pos p
        q = inv_hi[nid]                         # its hi-pos
        idx_perm = ((q % 128) * NGRP + q // 128).astype(np.int16)

        in_maps.append({
            "embed": embed_perm,
            "gidx_lo": _wrap16(idx_lo),
            "gidx_hi": _wrap16(idx_hi),
            "gidx_perm": _wrap16(idx_perm),
            "trend_in": trd.reshape(-1, 128).T.copy(),  # [128, CH_TOT]
        })
    plan.NLO_TOT = 128 * plan.NLO_CH
    plan.NHI_TOT = 128 * plan.NHI_CH
    return in_maps, plan


def emulate(in_maps, plan):
    """Numpy emulation of the device program (layout validation)."""
    outs = np.zeros((NCORES, HOPS, 128, NGRP * D), np.float32)
    agg = in_maps[0]["embed"].copy()

    def unwrap(wi, n):
        return wi[:16].T.reshape(-1)[:n].astype(np.int64)

    for h in range(HOPS):
        nxt = np.zeros((NP, D), np.float32)
        for c in range(NCORES):
            im = in_maps[c]
            ilo = unwrap(im["gidx_lo"], plan.NLO_TOT)
            ihi = unwrap(im["gidx_hi"], plan.NHI_TOT)
            iperm = unwrap(im["gidx_perm"], NLOCP)
            trd = im["trend_in"]                  # [128, CH_TOT]
            lo_view, hi_view = agg[0:RS], agg[HI_OFF:NP]

            comb = np.zeros((128, NGRP, D), np.float32)
            combh = np.zeros((128, NGRP, D), np.float32)
            for (chunks, runs, view, idx, ch_glob0, dstc) in (
                    (plan.hi_chunks, plan.hi_runs, hi_view, ihi,
                     plan.NLO_CH, combh),
                    (plan.lo_chunks, plan.lo_runs, lo_view, ilo, 0, comb)):
                for ci, (g0, g1, ch0, nch) in enumerate(chunks):
                    gt = view[idx[ch0 * 128:(ch0 + nch) * 128]].reshape(
                        nch, 128, D).transpose(1, 0, 2)   # [128, nch, D]
                    tw = trd[:, ch_glob0 + ch0: ch_glob0 + ch0 + nch]
                    gt = gt * tw[:, :, None]
                    for (goff, k, W, loff) in runs[ci]:
                        v = gt[:, loff:loff + k * W, :].reshape(128, k, W, D)
                        dstc[:, g0 + goff:g0 + goff + k, :] = v.sum(axis=2)
            # perm add
            rows = combh.reshape(128 * NGRP, D)   # row j*49+g
            comb = comb + rows[iperm].reshape(NGRP, 128, D).transpose(1, 0, 2)
            outs[c, h] = comb.reshape(128, NGRP * D)
            nxt[c * NLOCP:(c + 1) * NLOCP] = comb.reshape(128 * NGRP, D)
        agg = nxt
    return outs


def build(plan, repeat=1):
    nc = bacc.Bacc("TRN2", target_bir_lowering=False, debug=False,
                   num_devices=NCORES, num_swdge_queues=4)
    embed = nc.dram_tensor("embed", [NP, D], F32, kind="ExternalInput")
    gidx_lo = nc.dram_tensor("gidx_lo", [128, plan.NLO_TOT // 16], I16,
                             kind="ExternalInput")
    gidx_hi = nc.dram_tensor("gidx_hi", [128, plan.NHI_TOT // 16], I16,
                             kind="ExternalInput")
    gidx_perm = nc.dram_tensor("gidx_perm", [128, NLOCP // 16], I16,
                               kind="ExternalInput")
    trend_in = nc.dram_tensor("trend_in", [128, plan.CH_TOT], F32,
                              kind="ExternalInput")
    out3 = nc.dram_tensor("out3", [HOPS, 128, NGRP * D], F32,
                          kind="ExternalOutput")
    aggs = [embed] + [
        nc.dram_tensor(f"agg{h}", [NP, D], F32, addr_space="Shared")
        for h in range(1, HOPS)
    ]
    cc_in = [nc.dram_tensor(f"ccin{h}", [NLOCP, D], F32)
             for h in range(HOPS - 1)]
    hsc = [nc.dram_tensor(f"hsc{h}", [NLOCP, D], F32) for h in range(HOPS)]
    rg = [list(range(NCORES))]

    with tile.TileContext(nc) as tc:
        with (
            tc.tile_pool(name="meta", bufs=1) as meta,
            tc.tile_pool(name="gath", bufs=8) as gpool,
            tc.tile_pool(name="comb", bufs=1) as cpool,
            tc.tile_pool(name="combh", bufs=1) as hpool,
            tc.tile_pool(name="perm", bufs=1) as ppool,
        ):
            glo_sb = meta.tile([128, plan.NLO_TOT // 16], I16)
            nc.sync.dma_start(glo_sb[:], gidx_lo[:])
            ghi_sb = meta.tile([128, plan.NHI_TOT // 16], I16)
            nc.sync.dma_start(ghi_sb[:], gidx_hi[:])
            gpm_sb = meta.tile([128, NLOCP // 16], I16)
            nc.sync.dma_start(gpm_sb[:], gidx_perm[:])
            trend_sb = meta.tile([128, plan.CH_TOT, 1], F32)
            nc.sync.dma_start(
                trend_sb[:],
                trend_in[:].rearrange("p (a b) -> p a b", b=1))

            # queues 0-2 carry the bulk gathers; queue 3 is reserved for the
            # latency-critical tail (perm gather + final lo chunk) so their
            # DMAs do not queue behind bulk traffic.
            qn = [0]

            def next_q():
                q = qn[0] % 3
                qn[0] += 1
                return q

            def gather_chunk(chunks, view, idx_sb, ci, prep_sem=None, q=None):
                g0, g1, ch0, nch = chunks[ci]
                n = nch * 128
                gt = gpool.tile([128, plan.CHMAX, D], F32, tag="gt")
                nc.gpsimd.dma_gather(
                    gt[:, 0:nch, :], view,
                    idx_sb[:, ch0 * 8:(ch0 + nch) * 8],
                    n, n, D, single_packet=SINGLE_PACKET,
                    queue_num=next_q() if q is None else q,
                    prepare_only=prep_sem is not None, sem=prep_sem)
                return gt

            def combine_chunk(chunks, runs, ch_glob0, dstc, ci, gt):
                g0, g1, ch0, nch = chunks[ci]
                nc.vector.tensor_tensor(
                    gt[:, 0:nch, :], gt[:, 0:nch, :],
                    trend_sb[:, ch_glob0 + ch0:
                             ch_glob0 + ch0 + nch, :].broadcast_to(
                        [128, nch, D]),
                    mybir.AluOpType.mult)
                for (goff, k, W, loff) in runs[ci]:
                    nc.vector.tensor_reduce(
                        dstc[:, g0 + goff:g0 + goff + k, :],
                        gt[:, loff:loff + k * W, :].rearrange(
                            "p (k w) d -> p k d w", w=W),
                        mybir.AxisListType.X,
                        mybir.AluOpType.add)

            prepped = []
            for _rep in range(repeat):
              for h in range(HOPS):
                src = aggs[h].ap()
                lo_view = src[0:RS, :]
                hi_view = src[HI_OFF:NP, :]
                comb = cpool.tile([128, NGRP, D], F32, tag="comb")
                combh = hpool.tile([128, NGRP, D], F32, tag="combh")
                pt = ppool.tile([128, NGRP, D], F32, tag="pt")

                for ci in range(len(plan.hi_chunks)):
                    if ci < len(prepped):
                        gt = prepped[ci]
                    else:
                        gt = gather_chunk(plan.hi_chunks, hi_view, ghi_sb, ci)
                    combine_chunk(plan.hi_chunks, plan.hi_runs,
                                  plan.NLO_CH, combh, ci, gt)
                prepped = []
                nc.sync.dma_start(
                    hsc[h].ap().rearrange("a b -> (a b)").rearrange(
                        "(p x) -> p x", p=128),
                    combh[:].rearrange("p a b -> p (a b)"))

                nlo_ch = len(plan.lo_chunks)
                for ci in range(nlo_ch):
                    gt = gather_chunk(plan.lo_chunks, lo_view, glo_sb, ci,
                                      q=3 if ci == nlo_ch - 1 else None)
                    combine_chunk(plan.lo_chunks, plan.lo_runs,
                                  0, comb, ci, gt)
                    if ci == 0:
                        # perm gather: only needs the hsc dump; issuing it
                        # here lets its DMA overlap the remaining lo chunks
                        nc.gpsimd.dma_gather(
                            pt[:], hsc[h].ap(), gpm_sb[:], NLOCP, NLOCP, D,
                            single_packet=SINGLE_PACKET, queue_num=3)
                nc.vector.tensor_tensor(
                    comb[:], comb[:], pt[:], mybir.AluOpType.add)

                nc.sync.dma_start(
                    out3.ap()[h].rearrange("p x -> p x"),
                    comb[:].rearrange("p a b -> p (a b)"))
                if h < HOPS - 1:
                    nc.sync.dma_start(
                        cc_in[h].ap().rearrange("a b -> (a b)").rearrange(
                            "(p x) -> p x", p=128),
                        comb[:].rearrange("p a b -> p (a b)"))
                    nc.gpsimd.collective_compute(
                        "AllGather", mybir.AluOpType.bypass,
                        replica_groups=rg,
                        ins=[cc_in[h].ap().opt()],
                        outs=[aggs[h + 1].ap().opt()],
                    )
                    # prep next hop's first hi gathers during the AllGather:
                    # descriptor generation reads only the (static) index
                    # tiles, so it overlaps the collective; the trigger
                    # carries the data dependency on aggs[h+1].
                    nxt_hi = aggs[h + 1].ap()[HI_OFF:NP, :]
                    qs = []
                    for ci in range(min(N_PREP, len(plan.hi_chunks))):
                        sem = nc.alloc_semaphore(f"prep_dma_h{h}_{ci}")
                        q = next_q()
                        qs.append(q)
                        prepped.append(gather_chunk(
                            plan.hi_chunks, nxt_hi, ghi_sb, ci,
                            prep_sem=sem, q=q))
                    for q in qs:
                        nc.gpsimd.trigger_dma(count=None, queue_num=q)
    nc.compile()
    return nc


def assemble(embed, outs, order):
    """outs: per-core [HOPS, 128, NGRP*D] -> [N, HOPS+1, D]."""
    res = np.empty((N, HOPS + 1, D), np.float32)
    res[:, 0, :] = np.asarray(embed, dtype=np.float32)
    for c in range(NCORES):
        o3 = np.asarray(outs[c]).reshape(HOPS, 128, NGRP, D)
        vals = o3.transpose(0, 2, 1, 3).reshape(HOPS, NLOCP, D)
        valid = np.where(order[c] < NLOC)[0]
        nid = order[c][valid]
        for h in range(HOPS):
            res[c * NLOC + nid, h + 1, :] = vals[h, valid, :]
    return res


def run(embed, edge_index, trend, trace=False, trace_kwargs=None):
    in_maps, plan = preprocess(embed, edge_index, trend)
    nc = build(plan)
    r = run_bass_kernel_spmd(
        nc, in_maps, core_ids=list(range(NCORES)),
        trace=trace, **(trace_kwargs or {}))
    outs = [r.results[c]["out3"] for c in range(NCORES)]
    return assemble(embed, outs, plan.order), r


def kernel(embed, edge_index, trend):
    out, _ = run(embed, edge_index, trend)
    return out
